# revision 15
# baseline (speedup 1.0000x reference)
"""Bahdanau attention TRN2 kernel (B=8 data-parallel over 8 NeuronCores).

Separable approximation of the tanh score cube (see kernel_baseline.py for
the original 5-freq scheme):

  tanh(s) ~= alpha*s + sum_f c_f sin(w_f s),  s = a + b        (NF=4 + linear)

The alpha*a part is constant per query row -> softmax-invariant -> dropped.
The alpha*b part is a per-key bias, computed as zk[j] = (alpha*scale)^T @ kT
with 4 tiny N=1 matmuls and folded into the exp bias. The sin terms become
R=2*NF rank-1 products of per-side atoms sin(w_f x), cos(w_f x).

Atom argument prep per (freq, side), all f16 on DVE. HW probe: the ACT Sin
spline is valid to ~|x|<=3.8 rad (1.21*pi), which enables a shift trick that
needs NO abs op and shares one arg tile between sin and cos atoms:
  t0' = x*w/2pi + 0.125               tensor_scalar mult,add
  r   = (t0' + MAGIC) - MAGIC         tensor_scalar add,sub (fp32-internal
                                      magic-number round-to-nearest)
  d'  = t0' - r in [-.5,.5]           tensor_tensor sub
  (freq0 is direct: |t0'|<0.55, skip r/d')
  sin atoms = Sin(2pi*d' - pi/4) = sin(w x)   [args in [-3.93, 2.36]]
  cos atoms = Sin(2pi*d' + pi/4) = cos(w x)   [args in [-2.36, 3.93]]
ACT insts batched per (side, atom type, 2-freq group) for DVE/ACT/PE
pipelining; PSUM->SBUF projection casts ride ACT Copy (DVE is the spine).

Score matmuls (PE, fp16, PSUM accum over R terms), exp with mask+shift+
linear-k bias folded into a per-partition bias, ctx/Z matmuls jc-outer so
c_ps/z_ps accumulation pipelines with the exp chain, normalize on DVE
(f16 out), f16 output DMA (host upcasts to f32).
"""

import sys

if "/opt/trn_rl_repo" not in sys.path:
    sys.path.insert(0, "/opt/trn_rl_repo")

import math
import numpy as np

import concourse.bacc as bacc
import concourse.bass as bass
import concourse.tile as tile
import concourse.mybir as mybir

F32 = mybir.dt.float32
F16 = mybir.dt.float16
AF = mybir.ActivationFunctionType
OP = mybir.AluOpType

B, TQ, TV, D, U = 8, 512, 512, 512, 128
N_CORES = 8
TWO_PI = 2 * math.pi
PI_2 = math.pi / 2
MAGIC = 12582912.0  # 1.5 * 2^23: fp32 round-to-nearest via add/sub
EXP_SHIFT = -6.0
MASK_NEG = -30.0
N_WARM = 10  # PE warmup matmuls (HAM un-throttle during DMA lead-in)

# NF=4 + linear fit of tanh(a+b) on [-10.5, 10.5], gaussian-weighted
# (fit.py): tanh(s) ~= ALPHA*s + sum c_f sin(w_f s); e2e rel err ~1e-2.
FREQS = [
    (0.5345, 0.57626),
    (1.0755, 0.19653),
    (1.6678, 0.11397),
    (2.7215, 0.03656),
]
ALPHA = 0.17034
NF = len(FREQS)
R = 2 * NF
NC_EXPBIAS = R          # ctab cols R..R+3: exp bias per jc
NC_BSIN = R + 4         # ctab col R+4: -pi/4 (sin atom bias)
NC_BCOS = R + 5         # ctab col R+5: +pi/4 (cos atom bias)
CTAB_C = R + 6
SHIFT = 0.125           # recenters atom args into the Sin spline's range


def _emit(nc, debug=False):
    queryT = nc.dram_tensor("queryT", [128, 4, TQ], F16, kind="ExternalInput")
    keyT = nc.dram_tensor("keyT", [128, 4, TV], F16, kind="ExternalInput")
    value16 = nc.dram_tensor("value16", [128, 4, D], F16, kind="ExternalInput")
    wua = nc.dram_tensor("wua", [128, 8, U], F16, kind="ExternalInput")
    ctab = nc.dram_tensor("ctab", [128, CTAB_C], F32, kind="ExternalInput")
    ascale = nc.dram_tensor("ascale", [U, 1], F16, kind="ExternalInput")
    ctx16 = nc.dram_tensor("ctx16", [128, 4, D], F16, kind="ExternalOutput")
    env = dict(locals())

    with tile.TileContext(nc) as tc:
        _emit_body(nc, tc, env, debug)


def _emit_body(nc, tc, env, debug):
    queryT, keyT, value16, wua = (
        env["queryT"], env["keyT"], env["value16"], env["wua"]
    )
    ctab, ascale, ctx16 = env["ctab"], env["ascale"], env["ctx16"]

    with tc.tile_pool(name="const", bufs=1) as const:
        ctab_sb = const.tile([128, CTAB_C], F32, name="ctab_sb")
        ascale_sb = const.tile([U, 1], F16, name="ascale_sb")
        ones16 = const.tile([128, 1], F16, name="ones16")
        value_sb = const.tile([128, 4, D], F16, name="value_sb")
        qT16 = const.tile([U, TQ], F16, name="qT16")
        kT16 = const.tile([U, TV], F16, name="kT16")
        # atom argument tiles (shared by sin AND cos ACT insts), per side
        dk = const.tile([128, NF, TV], F16, name="dk")
        dq = const.tile([128, NF, TQ], F16, name="dq")
        # atom value tiles
        tk_s = const.tile([128, NF, TV], F16, name="tk_s")
        tk_c = const.tile([128, NF, TV], F16, name="tk_c")
        tq_s = const.tile([128, NF, TQ], F16, name="tq_s")
        tq_c = const.tile([128, NF, TQ], F16, name="tq_c")
        # wk = k-atoms * (scale_u * c_f) per term
        wkA = const.tile([128, NF, TV], F16, name="wkA")  # cos_k side
        wkB = const.tile([128, NF, TV], F16, name="wkB")  # sin_k side
        bias_sb = const.tile([128, 4], F32, name="bias_sb")
        wmT_sb = const.tile([128, 4, TQ], F16, name="wmT_sb")
        z_sb = const.tile([128, 4], F32, name="z_sb")
        zr_sb = const.tile([128, 4], F32, name="zr_sb")
        octx_sb = const.tile([128, 4, D], F16, name="octx_sb")
        scratch = const.tile([128, 2], F32, name="scratch")
        # f16 scratch for RR intermediates (r values per freq)
        rk = const.tile([128, NF, TV], F16, name="rk")
        rq = const.tile([128, NF, TQ], F16, name="rq")

        nc.vector.memset(ones16[:], 1.0)

        # ---- PE warmup during the DMA lead-in (HAM un-throttle) ----
        wsrc = const.tile([128, 512], F16, name="wsrc")
        nc.gpsimd.memset(wsrc[:], 0.0)
        # tiny dummy Sin: pin the sin table load into the DMA lead-in
        nc.scalar.activation(scratch[:, 0:1], wsrc[:, 0:1], AF.Sin, bias=0.0)
        with tc.tile_pool(name="warmps", bufs=1, space="PSUM") as warmps:
            wps = warmps.tile([128, 512], F32, name="wps")
            for _ in range(N_WARM):
                nc.tensor.matmul(wps[:], wsrc[:, :128], wsrc[:])

        # ---- input DMAs (host pre-shuffled to [128, ...] layouts) ----
        qT_r = queryT.ap()
        kT_r = keyT.ap()
        wua_r = wua.ap()

        with (
            tc.tile_pool(name="projin", bufs=1) as projin,
            tc.tile_pool(name="projps", bufs=1, space="PSUM") as projps,
            tc.tile_pool(name="zkps", bufs=1, space="PSUM") as zkps,
        ):
            qin = projin.tile([128, 4, TQ], F16, name="qin")
            kin = projin.tile([128, 4, TV], F16, name="kin")
            wua_sb = projin.tile([128, 8, U], F16, name="wua_sb")
            # spread input DMA issues over two engine queues so transfers
            # start right after the iram preamble; value16 is issued late
            # (from the scalar queue, after the atom insts) so it doesn't
            # steal bandwidth from kin/qin
            nc.gpsimd.dma_start(out=wua_sb[:], in_=wua_r)
            nc.sync.dma_start(out=kin[:, 0:2, :], in_=kT_r[:, 0:2, :])
            nc.sync.dma_start(out=kin[:, 2:4, :], in_=kT_r[:, 2:4, :])
            nc.gpsimd.dma_start(out=ctab_sb[:], in_=ctab.ap())
            nc.gpsimd.dma_start(out=ascale_sb[:], in_=ascale.ap())
            # qin transfer is serialized behind kin (1-elem copy creates a
            # WAW dep) so kin gets the full early DMA bandwidth: the k-side
            # projection->args->Sin chain is the kernel's critical spine
            nc.gpsimd.tensor_copy(out=qin[0:1, 0, 0:1], in_=kin[0:1, 3, 0:1])
            nc.sync.dma_start(out=qin[:], in_=qT_r)

            # ---- projections (PE); casts to f16 ride ACT Copy ----
            qT_ps = projps.tile([U, TQ], F32, name="qT_ps")
            kT_ps = projps.tile([U, TV], F32, name="kT_ps")
            for dc in range(4):
                nc.tensor.matmul(
                    kT_ps[:], wua_sb[:, 4 + dc, :], kin[:, dc, :],
                    start=(dc == 0), stop=(dc == 3),
                )
            nc.scalar.activation(kT16[:], kT_ps[:], AF.Copy, bias=0.0)
            # zk[j] = (alpha*scale)^T @ kT16 per jc chunk (linear-term bias)
            zk_ps = zkps.tile([128, 4], F32, name="zk_ps")
            for jc in range(4):
                nc.tensor.matmul(
                    zk_ps[:, jc : jc + 1],
                    kT16[:, jc * 128 : (jc + 1) * 128],
                    ascale_sb[:],
                    start=True, stop=True,
                )
            for dc in range(4):
                nc.tensor.matmul(
                    qT_ps[:], wua_sb[:, dc, :], qin[:, dc, :],
                    start=(dc == 0), stop=(dc == 3),
                )
            nc.vector.tensor_copy(out=qT16[:], in_=qT_ps[:])
            # exp bias = host mask/shift bias + alpha*zk
            nc.vector.tensor_tensor(
                out=bias_sb[:], in0=ctab_sb[:, R : R + 4], in1=zk_ps[:],
                op=OP.add,
            )

        # ---- atom args (DVE) + atoms (ACT) + score matmuls (PE) ----
        def rr_group(src, d_t, r_t, fpair):
            """Range-reduction chains for two freqs of one side (DVE).
            d' = (x*w/2pi + SHIFT) - round(x*w/2pi + SHIFT); the +-pi/4 ACT
            biases then give sin/cos within the Sin spline's +-3.9 range."""
            for f in fpair:
                w2p = FREQS[f][0] / TWO_PI
                dsl = d_t[:, f, :]
                nc.vector.tensor_scalar(
                    dsl, src[:], w2p, SHIFT, OP.mult, OP.add
                )
                if f != 0:
                    rsl = r_t[:, f, :]
                    nc.vector.tensor_scalar(
                        rsl, dsl, MAGIC, MAGIC, OP.add, OP.subtract
                    )
                    nc.vector.tensor_tensor(
                        out=dsl, in0=dsl, in1=rsl, op=OP.subtract
                    )

        def act_atoms(d_t, out_t, fpair, kind):
            """One batched ACT Sin over a 2-freq group of one side."""
            col = NC_BSIN if kind == "sin" else NC_BCOS
            f0 = fpair[0]
            n = len(fpair)
            nc.scalar.activation(
                out_t[:, f0 : f0 + n, :], d_t[:, f0 : f0 + n, :], AF.Sin,
                bias=ctab_sb[:, col : col + 1], scale=TWO_PI,
            )

        def score_mms(wk_t, tq_t, fpair, start_f, stop_f, jc_outer=False):
            order = (
                [(f, jc) for jc in range(4) for f in fpair]
                if jc_outer
                else [(f, jc) for f in fpair for jc in range(4)]
            )
            for f, jc in order:
                nc.tensor.matmul(
                    sT_ps[jc][:],
                    wk_t[:, f, jc * 128 : (jc + 1) * 128],
                    tq_t[:, f, :],
                    start=(f == start_f), stop=(f == stop_f),
                )

        with (
            tc.tile_pool(name="spsum", bufs=1, space="PSUM") as spsum,
            tc.tile_pool(name="kwps", bufs=1, space="PSUM") as kwps,
        ):
            sT_ps = [
                spsum.tile([128, TQ], F32, name=f"sT_ps{jc}") for jc in range(4)
            ]
            kw_ps = kwps.tile([128, 512], F32, name="kw_ps")

            G0, G1 = (0, 1), (2, 3)

            def wk_muls(wk_t, src_t, fpair, off):
                for f in fpair:
                    nc.vector.tensor_scalar(
                        wk_t[:, f, :], src_t[:, f, :],
                        ctab_sb[:, 2 * f + off : 2 * f + off + 1],
                        None, OP.mult,
                    )

            # zigzag k/q ACT schedule: score matmuls of each term group
            # start as soon as its (wk, q-atom) pair exists and drain
            # DURING the Sin stream, leaving only B23+exp+ctx as tail.
            def kw(anchor):
                # HAM keepwarm: a dummy MM data-anchored to `anchor` so the
                # scheduler places it at that point in the pipeline. The PE
                # clock gate re-throttles to 1.2 GHz after ~3.4us of low
                # activity; these keep density up through the atom phase.
                nc.tensor.matmul(kw_ps[:], anchor, wsrc[:],
                                 start=True, stop=True)

            rr_group(kT16, dk, rk, G0)                    # DVE
            kw(dk[:, 0, 0:128])
            kw(dk[:, 1, 0:128])
            act_atoms(dk, tk_c, G0, "cos")                # ACT 1
            rr_group(qT16, dq, rq, G0)                    # DVE
            kw(dq[:, 0, 0:128])
            kw(dq[:, 1, 0:128])
            kw(tk_c[:, 0, 0:128])
            kw(tk_c[:, 1, 0:128])
            wk_muls(wkA, tk_c, G0, 0)                     # DVE (after ACT 1)
            kw(wkA[:, 0, 0:128])
            act_atoms(dq, tq_s, G0, "sin")                # ACT 2
            score_mms(wkA, tq_s, G0, start_f=0, stop_f=-1)      # PE A01
            rr_group(kT16, dk, rk, G1)                    # DVE
            kw(dk[:, 2, 0:128])
            kw(dk[:, 3, 0:128])
            act_atoms(dk, tk_c, G1, "cos")                # ACT 3
            rr_group(qT16, dq, rq, G1)                    # DVE
            kw(dq[:, 2, 0:128])
            kw(dq[:, 3, 0:128])
            kw(tk_c[:, 2, 0:128])
            wk_muls(wkA, tk_c, G1, 0)                     # DVE (after ACT 3)
            kw(wkA[:, 2, 0:128])
            act_atoms(dq, tq_s, G1, "sin")                # ACT 4
            nc.tensor.matmul(kw_ps[:], dq[:, 1, 0:128], wsrc[:],
                             start=True, stop=True)       # keepwarm
            score_mms(wkA, tq_s, G1, start_f=-1, stop_f=-1)     # PE A23
            act_atoms(dk, tk_s, G0, "sin")                # ACT 5
            kw(tq_s[:, 0, 0:128])
            kw(tk_s[:, 0, 0:128])
            wk_muls(wkB, tk_s, G0, 1)                     # DVE (after ACT 5)
            # pin the value16 DMA behind the k-atom phase (1-elem copy makes
            # a WAW dep so it can't be hoisted into the kin/qin window)
            nc.vector.tensor_copy(
                out=value_sb[0:1, 0, 0:1], in_=tk_c[0:1, 0, 0:1]
            )
            act_atoms(dq, tq_c, G0, "cos")                # ACT 6
            nc.gpsimd.dma_start(out=value_sb[:], in_=value16.ap())
            score_mms(wkB, tq_c, G0, start_f=-1, stop_f=-1)     # PE B01
            act_atoms(dk, tk_s, G1, "sin")                # ACT 7
            kw(tq_c[:, 0, 0:128])
            kw(tk_s[:, 2, 0:128])
            wk_muls(wkB, tk_s, G1, 1)                     # DVE (after ACT 7)
            act_atoms(dq, tq_c, G1, "cos")                # ACT 8
            # dummy Exp pinned after the last Sin: prefetch exp table
            nc.scalar.activation(
                scratch[:, 1:2], tq_c[:, 0, 0:1], AF.Exp, bias=0.0
            )
            score_mms(wkB, tq_c, G1, start_f=-1, stop_f=3, jc_outer=True)
            # exp with mask+shift+linear bias per partition
            for jc in range(4):
                nc.scalar.activation(
                    wmT_sb[:, jc, :], sT_ps[jc][:], AF.Exp,
                    bias=bias_sb[:, jc : jc + 1],
                )

        # ---- tail: ctx = wmT^T @ value, Z = wmT^T @ ones, normalize ----
        with (
            tc.tile_pool(name="cpsum", bufs=1, space="PSUM") as cpsum,
            tc.tile_pool(name="zpsum", bufs=1, space="PSUM") as zpsum,
        ):
            c_ps = [cpsum.tile([128, D], F32, name=f"c_ps{ic}") for ic in range(4)]
            z_ps = [zpsum.tile([128, 1], F32, name=f"z_ps{ic}") for ic in range(4)]
            # jc-outer: ctx MMs for jc fire right after exp(jc); in the
            # final jc round the z MMs go first so the z->recip->normalize
            # chain overlaps the remaining ctx MMs
            for jc in range(4):
                for ic in range(4):
                    lhsT = wmT_sb[:, jc, ic * 128 : (ic + 1) * 128]
                    if jc == 3:
                        nc.tensor.matmul(
                            z_ps[ic][:], lhsT, ones16[:],
                            start=False, stop=True,
                        )
                        nc.tensor.matmul(
                            c_ps[ic][:], lhsT, value_sb[:, jc, :],
                            start=False, stop=True,
                        )
                    else:
                        nc.tensor.matmul(
                            c_ps[ic][:], lhsT, value_sb[:, jc, :],
                            start=(jc == 0), stop=False,
                        )
                        nc.tensor.matmul(
                            z_ps[ic][:], lhsT, ones16[:],
                            start=(jc == 0), stop=False,
                        )
            for ic in range(4):
                nc.vector.tensor_copy(
                    out=z_sb[:, ic : ic + 1], in_=z_ps[ic][:]
                )
            nc.vector.reciprocal(out=zr_sb[:], in_=z_sb[:])
            # normalize: ics 0,2 on ACT (Copy w/ per-partition scale), 1,3 on
            # DVE, so the two halves run concurrently; 2 batched out-DMAs
            for pair in ((0, 1), (2, 3)):
                a, b = pair
                nc.scalar.activation(
                    octx_sb[:, a, :], c_ps[a][:], AF.Copy,
                    scale=zr_sb[:, a : a + 1],
                )
                nc.vector.tensor_scalar(
                    octx_sb[:, b, :], c_ps[b][:],
                    zr_sb[:, b : b + 1], None, OP.mult,
                )
                nc.sync.dma_start(
                    out=ctx16.ap()[:, a : b + 1, :],
                    in_=octx_sb[:, a : b + 1, :],
                )


class _Runner:
    """Builds the Bass module once and holds a reusable jitted shard_map
    callable (mirrors concourse.bass2jax.run_bass_via_pjrt, but persistent
    so repeat calls don't re-jit/re-compile)."""

    def __init__(self, debug=False):
        import jax
        from concourse.bass2jax import install_neuronx_cc_hook, _bass_exec_p
        from jax.experimental.shard_map import shard_map
        from jax.sharding import Mesh, PartitionSpec

        self.jax = jax
        nc = bacc.Bacc(
            "TRN2", target_bir_lowering=False, debug=False,
            enable_asserts=False, num_devices=N_CORES,
            enable_partition_id=False,
        )
        _emit(nc, debug=debug)
        nc.compile()
        self.nc = nc

        install_neuronx_cc_hook()
        in_names, out_names, out_avals = [], [], []
        for alloc in nc.m.functions[0].allocations:
            if not isinstance(alloc, mybir.MemoryLocationSet):
                continue
            name = alloc.memorylocations[0].name
            if alloc.kind == "ExternalInput":
                in_names.append(name)
            elif alloc.kind == "ExternalOutput":
                out_names.append(name)
                out_avals.append(
                    jax.core.ShapedArray(
                        tuple(alloc.tensor_shape), mybir.dt.np(alloc.dtype)
                    )
                )
        assert nc.partition_id_tensor is None
        self.in_names = in_names
        self.out_names = out_names
        self.out_avals = out_avals
        n_params = len(in_names)
        n_outs = len(out_names)
        all_names = tuple(in_names + out_names)

        def _body(*args):
            outs = _bass_exec_p.bind(
                *args,
                out_avals=tuple(out_avals),
                in_names=all_names,
                out_names=tuple(out_names),
                lowering_input_output_aliases=(),
                sim_require_finite=True,
                sim_require_nnan=True,
                nc=nc,
            )
            return tuple(outs)

        devices = jax.devices()[:N_CORES]
        self.mesh = Mesh(np.asarray(devices), ("core",))
        self.pspec = PartitionSpec("core")
        in_specs = (self.pspec,) * (n_params + n_outs)
        out_specs = (self.pspec,) * n_outs
        donate = tuple(range(n_params, n_params + n_outs))
        self.sharded = jax.jit(
            shard_map(
                _body, mesh=self.mesh, in_specs=in_specs, out_specs=out_specs,
                check_rep=False,
            ),
            donate_argnums=donate,
            keep_unused=True,
        )

    def concat_inputs(self, in_maps):
        return [
            np.concatenate([np.asarray(m[name]) for m in in_maps], axis=0)
            for name in self.in_names
        ]

    def fresh_zeros(self):
        return [
            np.zeros((N_CORES * a.shape[0], *a.shape[1:]), a.dtype)
            for a in self.out_avals
        ]

    def run(self, in_maps):
        out_arrs = self.sharded(*self.concat_inputs(in_maps), *self.fresh_zeros())
        i = self.out_names.index("ctx16")
        a = self.out_avals[i]
        out = np.asarray(out_arrs[i]).reshape(N_CORES, *a.shape)
        # [B, p, c, d] -> [B, c*128+p, d]
        return (
            out.transpose(0, 2, 1, 3)
            .reshape(N_CORES, TQ, D)
            .astype(np.float32)
        )


_runner = None


def _get_runner():
    global _runner
    if _runner is None:
        _runner = _Runner()
    return _runner


def _make_in_maps(query, key, value, mask, Wa, Ua, scale):
    query = np.asarray(query, dtype=np.float32)
    key = np.asarray(key, dtype=np.float32)
    value = np.asarray(value, dtype=np.float32)
    mask = np.asarray(mask)
    Wa = np.ascontiguousarray(np.asarray(Wa, dtype=np.float32))
    Ua = np.ascontiguousarray(np.asarray(Ua, dtype=np.float32))
    scale = np.ascontiguousarray(np.asarray(scale, dtype=np.float32))
    wua = np.concatenate([Wa, Ua], axis=0).astype(np.float16)
    # pre-shuffled DMA layouts: [p, c, i] with source row = c*128 + p
    wua_s = np.ascontiguousarray(
        wua.reshape(8, 128, U).transpose(1, 0, 2)
    )
    qs = [
        np.ascontiguousarray(
            query[b].T.astype(np.float16).reshape(4, 128, TQ).transpose(1, 0, 2)
        )
        for b in range(B)
    ]
    ks = [
        np.ascontiguousarray(
            key[b].T.astype(np.float16).reshape(4, 128, TV).transpose(1, 0, 2)
        )
        for b in range(B)
    ]
    vs = [
        np.ascontiguousarray(
            value[b].astype(np.float16).reshape(4, 128, D).transpose(1, 0, 2)
        )
        for b in range(B)
    ]
    ctab = np.zeros((128, CTAB_C), dtype=np.float32)
    for f, (_w, c) in enumerate(FREQS):
        ctab[:, 2 * f] = scale * c
        ctab[:, 2 * f + 1] = scale * c
    ctab[:, NC_BSIN] = -math.pi / 4
    ctab[:, NC_BCOS] = math.pi / 4
    ascale16 = (ALPHA * scale).astype(np.float16).reshape(U, 1)
    in_maps = []
    for b in range(B):
        mb = np.where(mask[b], EXP_SHIFT, EXP_SHIFT + MASK_NEG).astype(np.float32)
        ct = ctab.copy()
        ct[:, R : R + 4] = mb.reshape(4, 128).T
        in_maps.append(
            {
                "queryT": qs[b],
                "keyT": ks[b],
                "value16": vs[b],
                "wua": wua_s,
                "ctab": ct,
                "ascale": ascale16,
            }
        )
    return in_maps


def kernel(query, key, value, mask, Wa, Ua, scale):
    r = _get_runner()
    in_maps = _make_in_maps(query, key, value, mask, Wa, Ua, scale)
    return r.run(in_maps)


# revision 16
# speedup vs baseline: 1.0608x; 1.0608x over previous
"""Bahdanau attention TRN2 kernel (B=8 data-parallel over 8 NeuronCores).

Separable approximation of the tanh score cube (see kernel_baseline.py for
the original 5-freq scheme):

  tanh(s) ~= alpha*s + sum_f c_f sin(w_f s),  s = a + b        (NF=4 + linear)

The alpha*a part is constant per query row -> softmax-invariant -> dropped.
The alpha*b part is a per-key bias, computed as zk[j] = (alpha*scale)^T @ kT
with 4 tiny N=1 matmuls and folded into the exp bias. The sin terms become
R=2*NF rank-1 products of per-side atoms sin(w_f x), cos(w_f x).

Atom argument prep per (freq, side), all f16 on DVE. HW probe: the ACT Sin
spline is valid to ~|x|<=3.8 rad (1.21*pi), which enables a shift trick that
needs NO abs op and shares one arg tile between sin and cos atoms:
  t0' = x*w/2pi + 0.125               tensor_scalar mult,add
  r   = (t0' + MAGIC) - MAGIC         tensor_scalar add,sub (fp32-internal
                                      magic-number round-to-nearest)
  d'  = t0' - r in [-.5,.5]           tensor_tensor sub
  (freq0 is direct: |t0'|<0.55, skip r/d')
  sin atoms = Sin(2pi*d' - pi/4) = sin(w x)   [args in [-3.93, 2.36]]
  cos atoms = Sin(2pi*d' + pi/4) = cos(w x)   [args in [-2.36, 3.93]]
ACT insts batched per (side, atom type, 2-freq group) for DVE/ACT/PE
pipelining; PSUM->SBUF projection casts ride ACT Copy (DVE is the spine).

Score matmuls (PE, fp16, PSUM accum over R terms), exp with mask+shift+
linear-k bias folded into a per-partition bias, ctx/Z matmuls jc-outer so
c_ps/z_ps accumulation pipelines with the exp chain, normalize on DVE
(f16 out), f16 output DMA (host upcasts to f32).
"""

import sys

if "/opt/trn_rl_repo" not in sys.path:
    sys.path.insert(0, "/opt/trn_rl_repo")

import math
import numpy as np

import concourse.bacc as bacc
import concourse.bass as bass
import concourse.tile as tile
import concourse.mybir as mybir

F32 = mybir.dt.float32
F16 = mybir.dt.float16
AF = mybir.ActivationFunctionType
OP = mybir.AluOpType

B, TQ, TV, D, U = 8, 512, 512, 512, 128
N_CORES = 8
TWO_PI = 2 * math.pi
PI_2 = math.pi / 2
MAGIC = 12582912.0  # 1.5 * 2^23: fp32 round-to-nearest via add/sub
EXP_SHIFT = -6.0
MASK_NEG = -30.0
N_WARM = 6  # PE warmup matmuls (HAM un-throttle during DMA lead-in)

# NF=4 + linear fit of tanh(a+b) on [-10.5, 10.5], gaussian-weighted
# (fit.py): tanh(s) ~= ALPHA*s + sum c_f sin(w_f s); e2e rel err ~1e-2.
FREQS = [
    (0.5345, 0.57626),
    (1.0755, 0.19653),
    (1.6678, 0.11397),
    (2.7215, 0.03656),
]
ALPHA = 0.17034
NF = len(FREQS)
R = 2 * NF
NC_EXPBIAS = R          # ctab cols R..R+3: exp bias per jc
NC_BSIN = R + 4         # ctab col R+4: -pi/4 (sin atom bias)
NC_BCOS = R + 5         # ctab col R+5: +pi/4 (cos atom bias)
CTAB_C = R + 6
SHIFT = 0.125           # recenters atom args into the Sin spline's range


def _emit(nc, debug=False):
    queryT = nc.dram_tensor("queryT", [D, TQ], F16, kind="ExternalInput")
    keyT = nc.dram_tensor("keyT", [D, TV], F16, kind="ExternalInput")
    value16 = nc.dram_tensor("value16", [TV, D], F16, kind="ExternalInput")
    wua = nc.dram_tensor("wua", [2 * D, U], F16, kind="ExternalInput")
    ctab = nc.dram_tensor("ctab", [128, CTAB_C], F32, kind="ExternalInput")
    ascale = nc.dram_tensor("ascale", [U, 1], F16, kind="ExternalInput")
    ctx16 = nc.dram_tensor("ctx16", [TQ, D], F16, kind="ExternalOutput")
    env = dict(locals())

    with tile.TileContext(nc) as tc:
        _emit_body(nc, tc, env, debug)


def _emit_body(nc, tc, env, debug):
    queryT, keyT, value16, wua = (
        env["queryT"], env["keyT"], env["value16"], env["wua"]
    )
    ctab, ascale, ctx16 = env["ctab"], env["ascale"], env["ctx16"]

    with tc.tile_pool(name="const", bufs=1) as const:
        ctab_sb = const.tile([128, CTAB_C], F32, name="ctab_sb")
        ascale_sb = const.tile([U, 1], F16, name="ascale_sb")
        ones16 = const.tile([128, 1], F16, name="ones16")
        value_sb = const.tile([128, 4, D], F16, name="value_sb")
        qT16 = const.tile([U, TQ], F16, name="qT16")
        kT16 = const.tile([U, TV], F16, name="kT16")
        # atom argument tiles (shared by sin AND cos ACT insts), per side
        dk = const.tile([128, NF, TV], F16, name="dk")
        dq = const.tile([128, NF, TQ], F16, name="dq")
        # atom value tiles
        tk_s = const.tile([128, NF, TV], F16, name="tk_s")
        tk_c = const.tile([128, NF, TV], F16, name="tk_c")
        tq_s = const.tile([128, NF, TQ], F16, name="tq_s")
        tq_c = const.tile([128, NF, TQ], F16, name="tq_c")
        # wk = k-atoms * (scale_u * c_f) per term
        wkA = const.tile([128, NF, TV], F16, name="wkA")  # cos_k side
        wkB = const.tile([128, NF, TV], F16, name="wkB")  # sin_k side
        bias_sb = const.tile([128, 4], F32, name="bias_sb")
        wmT_sb = const.tile([128, 4, TQ], F16, name="wmT_sb")
        z_sb = const.tile([128, 4], F32, name="z_sb")
        zr_sb = const.tile([128, 4], F32, name="zr_sb")
        octx_sb = const.tile([128, 4, D], F16, name="octx_sb")
        scratch = const.tile([128, 2], F32, name="scratch")
        # f16 scratch for RR intermediates (r values per freq)
        rk = const.tile([128, NF, TV], F16, name="rk")
        rq = const.tile([128, NF, TQ], F16, name="rq")

        nc.vector.memset(ones16[:], 1.0)

        # ---- PE warmup during the DMA lead-in (HAM un-throttle) ----
        wsrc = const.tile([128, 512], F16, name="wsrc")
        nc.gpsimd.memset(wsrc[:], 0.0)
        # tiny dummy Sin: pin the sin table load into the DMA lead-in
        nc.scalar.activation(scratch[:, 0:1], wsrc[:, 0:1], AF.Sin, bias=0.0)
        with tc.tile_pool(name="warmps", bufs=1, space="PSUM") as warmps:
            wps = warmps.tile([128, 512], F32, name="wps")
            for _ in range(N_WARM):
                nc.tensor.matmul(wps[:], wsrc[:, :128], wsrc[:])

        # ---- input DMAs (order = arrival order; value16 last) ----
        qT_r = queryT.ap().rearrange("(c p) i -> p c i", p=128)
        kT_r = keyT.ap().rearrange("(c p) i -> p c i", p=128)
        wua_r = wua.ap().rearrange("(s c p) u -> p (s c) u", p=128, c=4)

        with (
            tc.tile_pool(name="projin", bufs=1) as projin,
            tc.tile_pool(name="projps", bufs=1, space="PSUM") as projps,
            tc.tile_pool(name="zkps", bufs=1, space="PSUM") as zkps,
        ):
            qin = projin.tile([128, 4, TQ], F16, name="qin")
            kin = projin.tile([128, 4, TV], F16, name="kin")
            wua_sb = projin.tile([128, 8, U], F16, name="wua_sb")
            # spread input DMA issues over two engine queues so transfers
            # start right after the iram preamble; value16 is issued late
            # (from the scalar queue, after the atom insts) so it doesn't
            # steal bandwidth from kin/qin
            nc.sync.dma_start(out=kin[:, 0:2, :], in_=kT_r[:, 0:2, :])
            nc.sync.dma_start(out=kin[:, 2:4, :], in_=kT_r[:, 2:4, :])
            nc.gpsimd.dma_start(out=wua_sb[:], in_=wua_r)
            nc.gpsimd.dma_start(out=ctab_sb[:], in_=ctab.ap())
            nc.gpsimd.dma_start(out=ascale_sb[:], in_=ascale.ap())
            # qin transfer is serialized behind kin (1-elem copy creates a
            # WAW dep) so kin gets the full early DMA bandwidth: the k-side
            # projection->args->Sin chain is the kernel's critical spine
            nc.gpsimd.tensor_copy(out=qin[0:1, 0, 0:1], in_=kin[0:1, 3, 0:1])
            nc.sync.dma_start(out=qin[:], in_=qT_r)

            # ---- projections (PE); casts to f16 ride ACT Copy ----
            qT_ps = projps.tile([U, TQ], F32, name="qT_ps")
            kT_ps = projps.tile([U, TV], F32, name="kT_ps")
            for dc in range(4):
                nc.tensor.matmul(
                    kT_ps[:], wua_sb[:, 4 + dc, :], kin[:, dc, :],
                    start=(dc == 0), stop=(dc == 3),
                )
            nc.scalar.activation(kT16[:], kT_ps[:], AF.Copy, bias=0.0)
            # zk[j] = (alpha*scale)^T @ kT16 per jc chunk (linear-term bias)
            zk_ps = zkps.tile([128, 4], F32, name="zk_ps")
            for jc in range(4):
                nc.tensor.matmul(
                    zk_ps[:, jc : jc + 1],
                    kT16[:, jc * 128 : (jc + 1) * 128],
                    ascale_sb[:],
                    start=True, stop=True,
                )
            for dc in range(4):
                nc.tensor.matmul(
                    qT_ps[:], wua_sb[:, dc, :], qin[:, dc, :],
                    start=(dc == 0), stop=(dc == 3),
                )
            nc.scalar.activation(qT16[:], qT_ps[:], AF.Copy, bias=0.0)
            # exp bias = host mask/shift bias + alpha*zk
            nc.vector.tensor_tensor(
                out=bias_sb[:], in0=ctab_sb[:, R : R + 4], in1=zk_ps[:],
                op=OP.add,
            )

        # ---- atom args (DVE) + atoms (ACT) + score matmuls (PE) ----
        def rr_group(src, d_t, r_t, fpair):
            """Range-reduction chains for two freqs of one side (DVE).
            d' = (x*w/2pi + SHIFT) - round(x*w/2pi + SHIFT); the +-pi/4 ACT
            biases then give sin/cos within the Sin spline's +-3.9 range."""
            for f in fpair:
                w2p = FREQS[f][0] / TWO_PI
                dsl = d_t[:, f, :]
                nc.vector.tensor_scalar(
                    dsl, src[:], w2p, SHIFT, OP.mult, OP.add
                )
                if f != 0:
                    rsl = r_t[:, f, :]
                    nc.vector.tensor_scalar(
                        rsl, dsl, MAGIC, MAGIC, OP.add, OP.subtract
                    )
                    nc.vector.tensor_tensor(
                        out=dsl, in0=dsl, in1=rsl, op=OP.subtract
                    )

        def act_atoms(d_t, out_t, fpair, kind):
            """One batched ACT Sin over a 2-freq group of one side."""
            col = NC_BSIN if kind == "sin" else NC_BCOS
            f0 = fpair[0]
            n = len(fpair)
            nc.scalar.activation(
                out_t[:, f0 : f0 + n, :], d_t[:, f0 : f0 + n, :], AF.Sin,
                bias=ctab_sb[:, col : col + 1], scale=TWO_PI,
            )

        def score_mms(wk_t, tq_t, fpair, start_f, stop_f, jc_outer=False):
            order = (
                [(f, jc) for jc in range(4) for f in fpair]
                if jc_outer
                else [(f, jc) for f in fpair for jc in range(4)]
            )
            for f, jc in order:
                nc.tensor.matmul(
                    sT_ps[jc][:],
                    wk_t[:, f, jc * 128 : (jc + 1) * 128],
                    tq_t[:, f, :],
                    start=(f == start_f), stop=(f == stop_f),
                )

        with (
            tc.tile_pool(name="spsum", bufs=1, space="PSUM") as spsum,
            tc.tile_pool(name="kwps", bufs=1, space="PSUM") as kwps,
        ):
            sT_ps = [
                spsum.tile([128, TQ], F32, name=f"sT_ps{jc}") for jc in range(4)
            ]
            kw_ps = kwps.tile([128, 512], F32, name="kw_ps")

            G0, G1 = (0, 1), (2, 3)
            # DVE arg order: k01, k23, q01, wkA01, q23, wkA23 (then wkB)
            rr_group(kT16, dk, rk, G0)
            act_atoms(dk, tk_c, G0, "cos")
            rr_group(kT16, dk, rk, G1)
            act_atoms(dk, tk_c, G1, "cos")
            rr_group(qT16, dq, rq, G0)
            for f in G0:
                nc.vector.tensor_scalar(
                    wkA[:, f, :], tk_c[:, f, :],
                    ctab_sb[:, 2 * f : 2 * f + 1], None, OP.mult,
                )
            act_atoms(dq, tq_s, G0, "sin")
            score_mms(wkA, tq_s, G0, start_f=0, stop_f=-1)
            rr_group(qT16, dq, rq, G1)
            for f in G1:
                nc.vector.tensor_scalar(
                    wkA[:, f, :], tk_c[:, f, :],
                    ctab_sb[:, 2 * f : 2 * f + 1], None, OP.mult,
                )
            # keepwarm MMs anchored on fresh args (prevent HAM re-throttle)
            nc.tensor.matmul(kw_ps[:], dk[:, 1, 0:128], wsrc[:],
                             start=True, stop=True)
            nc.tensor.matmul(kw_ps[:], dk[:, 3, 0:128], wsrc[:],
                             start=True, stop=True)
            nc.tensor.matmul(kw_ps[:], dq[:, 1, 0:128], wsrc[:],
                             start=True, stop=True)
            act_atoms(dq, tq_s, G1, "sin")
            score_mms(wkA, tq_s, G1, start_f=-1, stop_f=-1)
            # pin the value16 DMA behind the k-atom phase: the 1-elem copy
            # below depends on tk_c, and the DMA (WAW on value_sb) then
            # cannot be hoisted into the kin/qin transfer window
            nc.vector.tensor_copy(
                out=value_sb[0:1, 0, 0:1], in_=tk_c[0:1, 0, 0:1]
            )
            act_atoms(dk, tk_s, G0, "sin")
            act_atoms(dk, tk_s, G1, "sin")
            for f in range(NF):
                nc.vector.tensor_scalar(
                    wkB[:, f, :], tk_s[:, f, :],
                    ctab_sb[:, 2 * f + 1 : 2 * f + 2], None, OP.mult,
                )
            act_atoms(dq, tq_c, G0, "cos")
            score_mms(wkB, tq_c, G0, start_f=-1, stop_f=-1)
            act_atoms(dq, tq_c, G1, "cos")
            # value16 DMA: issued from the scalar queue, gated by the
            # tk_c-dependent 1-elem write above
            nc.scalar.dma_start(
                out=value_sb[:],
                in_=value16.ap().rearrange("(c p) d -> p c d", p=128),
            )
            # dummy Exp pinned after the last Sin: prefetch exp table
            nc.scalar.activation(
                scratch[:, 1:2], tq_c[:, 0, 0:1], AF.Exp, bias=0.0
            )
            score_mms(wkB, tq_c, G1, start_f=-1, stop_f=3, jc_outer=True)
            # exp with mask+shift+linear bias per partition
            for jc in range(4):
                nc.scalar.activation(
                    wmT_sb[:, jc, :], sT_ps[jc][:], AF.Exp,
                    bias=bias_sb[:, jc : jc + 1],
                )

        # ---- tail: ctx = wmT^T @ value, Z = wmT^T @ ones, normalize ----
        with (
            tc.tile_pool(name="cpsum", bufs=1, space="PSUM") as cpsum,
            tc.tile_pool(name="zpsum", bufs=1, space="PSUM") as zpsum,
        ):
            c_ps = [cpsum.tile([128, D], F32, name=f"c_ps{ic}") for ic in range(4)]
            z_ps = [zpsum.tile([128, 1], F32, name=f"z_ps{ic}") for ic in range(4)]
            # jc-outer: ctx MMs for jc fire right after exp(jc); in the
            # final jc round the z MMs go first so the z->recip->normalize
            # chain overlaps the remaining ctx MMs
            for jc in range(4):
                for ic in range(4):
                    lhsT = wmT_sb[:, jc, ic * 128 : (ic + 1) * 128]
                    if jc == 3:
                        nc.tensor.matmul(
                            z_ps[ic][:], lhsT, ones16[:],
                            start=False, stop=True,
                        )
                        nc.tensor.matmul(
                            c_ps[ic][:], lhsT, value_sb[:, jc, :],
                            start=False, stop=True,
                        )
                    else:
                        nc.tensor.matmul(
                            c_ps[ic][:], lhsT, value_sb[:, jc, :],
                            start=(jc == 0), stop=False,
                        )
                        nc.tensor.matmul(
                            z_ps[ic][:], lhsT, ones16[:],
                            start=(jc == 0), stop=False,
                        )
            for ic in range(4):
                nc.vector.tensor_copy(
                    out=z_sb[:, ic : ic + 1], in_=z_ps[ic][:]
                )
            nc.vector.reciprocal(out=zr_sb[:], in_=z_sb[:])
            # normalize: ics 0,2 on ACT (Copy w/ per-partition scale), 1,3 on
            # DVE, so the two halves run concurrently; 2 batched out-DMAs
            for pair in ((0, 1), (2, 3)):
                a, b = pair
                nc.scalar.activation(
                    octx_sb[:, a, :], c_ps[a][:], AF.Copy,
                    scale=zr_sb[:, a : a + 1],
                )
                nc.vector.tensor_scalar(
                    octx_sb[:, b, :], c_ps[b][:],
                    zr_sb[:, b : b + 1], None, OP.mult,
                )
                nc.sync.dma_start(
                    out=ctx16.ap().rearrange(
                        "(c p) d -> p c d", p=128
                    )[:, a : b + 1, :],
                    in_=octx_sb[:, a : b + 1, :],
                )


class _Runner:
    """Builds the Bass module once and holds a reusable jitted shard_map
    callable (mirrors concourse.bass2jax.run_bass_via_pjrt, but persistent
    so repeat calls don't re-jit/re-compile)."""

    def __init__(self, debug=False):
        import jax
        from concourse.bass2jax import install_neuronx_cc_hook, _bass_exec_p
        from jax.experimental.shard_map import shard_map
        from jax.sharding import Mesh, PartitionSpec

        self.jax = jax
        nc = bacc.Bacc(
            "TRN2", target_bir_lowering=False, debug=False,
            enable_asserts=False, num_devices=N_CORES,
            enable_partition_id=False,
        )
        _emit(nc, debug=debug)
        nc.compile()
        self.nc = nc

        install_neuronx_cc_hook()
        in_names, out_names, out_avals = [], [], []
        for alloc in nc.m.functions[0].allocations:
            if not isinstance(alloc, mybir.MemoryLocationSet):
                continue
            name = alloc.memorylocations[0].name
            if alloc.kind == "ExternalInput":
                in_names.append(name)
            elif alloc.kind == "ExternalOutput":
                out_names.append(name)
                out_avals.append(
                    jax.core.ShapedArray(
                        tuple(alloc.tensor_shape), mybir.dt.np(alloc.dtype)
                    )
                )
        assert nc.partition_id_tensor is None
        self.in_names = in_names
        self.out_names = out_names
        self.out_avals = out_avals
        n_params = len(in_names)
        n_outs = len(out_names)
        all_names = tuple(in_names + out_names)

        def _body(*args):
            outs = _bass_exec_p.bind(
                *args,
                out_avals=tuple(out_avals),
                in_names=all_names,
                out_names=tuple(out_names),
                lowering_input_output_aliases=(),
                sim_require_finite=True,
                sim_require_nnan=True,
                nc=nc,
            )
            return tuple(outs)

        devices = jax.devices()[:N_CORES]
        self.mesh = Mesh(np.asarray(devices), ("core",))
        self.pspec = PartitionSpec("core")
        in_specs = (self.pspec,) * (n_params + n_outs)
        out_specs = (self.pspec,) * n_outs
        donate = tuple(range(n_params, n_params + n_outs))
        self.sharded = jax.jit(
            shard_map(
                _body, mesh=self.mesh, in_specs=in_specs, out_specs=out_specs,
                check_rep=False,
            ),
            donate_argnums=donate,
            keep_unused=True,
        )

    def concat_inputs(self, in_maps):
        return [
            np.concatenate([np.asarray(m[name]) for m in in_maps], axis=0)
            for name in self.in_names
        ]

    def fresh_zeros(self):
        return [
            np.zeros((N_CORES * a.shape[0], *a.shape[1:]), a.dtype)
            for a in self.out_avals
        ]

    def run(self, in_maps):
        out_arrs = self.sharded(*self.concat_inputs(in_maps), *self.fresh_zeros())
        i = self.out_names.index("ctx16")
        a = self.out_avals[i]
        return (
            np.asarray(out_arrs[i])
            .reshape(N_CORES, *a.shape)
            .astype(np.float32)
        )


_runner = None


def _get_runner():
    global _runner
    if _runner is None:
        _runner = _Runner()
    return _runner


def _make_in_maps(query, key, value, mask, Wa, Ua, scale):
    query = np.asarray(query, dtype=np.float32)
    key = np.asarray(key, dtype=np.float32)
    value = np.asarray(value, dtype=np.float32)
    mask = np.asarray(mask)
    Wa = np.ascontiguousarray(np.asarray(Wa, dtype=np.float32))
    Ua = np.ascontiguousarray(np.asarray(Ua, dtype=np.float32))
    scale = np.ascontiguousarray(np.asarray(scale, dtype=np.float32))
    wua = np.concatenate([Wa, Ua], axis=0).astype(np.float16)
    ctab = np.zeros((128, CTAB_C), dtype=np.float32)
    for f, (_w, c) in enumerate(FREQS):
        ctab[:, 2 * f] = scale * c
        ctab[:, 2 * f + 1] = scale * c
    ctab[:, NC_BSIN] = -math.pi / 4
    ctab[:, NC_BCOS] = math.pi / 4
    ascale16 = (ALPHA * scale).astype(np.float16).reshape(U, 1)
    in_maps = []
    for b in range(B):
        mb = np.where(mask[b], EXP_SHIFT, EXP_SHIFT + MASK_NEG).astype(np.float32)
        ct = ctab.copy()
        ct[:, R : R + 4] = mb.reshape(4, 128).T
        in_maps.append(
            {
                "queryT": np.ascontiguousarray(query[b].T).astype(np.float16),
                "keyT": np.ascontiguousarray(key[b].T).astype(np.float16),
                "value16": value[b].astype(np.float16),
                "wua": wua,
                "ctab": ct,
                "ascale": ascale16,
            }
        )
    return in_maps


def kernel(query, key, value, mask, Wa, Ua, scale):
    r = _get_runner()
    in_maps = _make_in_maps(query, key, value, mask, Wa, Ua, scale)
    return r.run(in_maps)


# revision 17
# speedup vs baseline: 1.0646x; 1.0036x over previous
"""Bahdanau attention TRN2 kernel (B=8 data-parallel over 8 NeuronCores).

Separable approximation of the tanh score cube (see kernel_baseline.py for
the original 5-freq scheme):

  tanh(s) ~= alpha*s + sum_f c_f sin(w_f s),  s = a + b        (NF=4 + linear)

The alpha*a part is constant per query row -> softmax-invariant -> dropped.
The alpha*b part is a per-key bias, computed as zk[j] = (alpha*scale)^T @ kT
with 4 tiny N=1 matmuls and folded into the exp bias. The sin terms become
R=2*NF rank-1 products of per-side atoms sin(w_f x), cos(w_f x).

Atom argument prep per (freq, side), all f16 on DVE. HW probe: the ACT Sin
spline is valid to ~|x|<=3.8 rad (1.21*pi), which enables a shift trick that
needs NO abs op and shares one arg tile between sin and cos atoms:
  t0' = x*w/2pi + 0.125               tensor_scalar mult,add
  r   = (t0' + MAGIC) - MAGIC         tensor_scalar add,sub (fp32-internal
                                      magic-number round-to-nearest)
  d'  = t0' - r in [-.5,.5]           tensor_tensor sub
  (freq0 is direct: |t0'|<0.55, skip r/d')
  sin atoms = Sin(2pi*d' - pi/4) = sin(w x)   [args in [-3.93, 2.36]]
  cos atoms = Sin(2pi*d' + pi/4) = cos(w x)   [args in [-2.36, 3.93]]
ACT insts batched per (side, atom type, 2-freq group) for DVE/ACT/PE
pipelining; PSUM->SBUF projection casts ride ACT Copy (DVE is the spine).

Score matmuls (PE, fp16, PSUM accum over R terms), exp with mask+shift+
linear-k bias folded into a per-partition bias, ctx/Z matmuls jc-outer so
c_ps/z_ps accumulation pipelines with the exp chain, normalize on DVE
(f16 out), f16 output DMA (host upcasts to f32).
"""

import sys

if "/opt/trn_rl_repo" not in sys.path:
    sys.path.insert(0, "/opt/trn_rl_repo")

import math
import numpy as np

import concourse.bacc as bacc
import concourse.bass as bass
import concourse.tile as tile
import concourse.mybir as mybir

F32 = mybir.dt.float32
F16 = mybir.dt.float16
AF = mybir.ActivationFunctionType
OP = mybir.AluOpType

B, TQ, TV, D, U = 8, 512, 512, 512, 128
N_CORES = 8
TWO_PI = 2 * math.pi
PI_2 = math.pi / 2
MAGIC = 12582912.0  # 1.5 * 2^23: fp32 round-to-nearest via add/sub
EXP_SHIFT = -6.0
MASK_NEG = -30.0
N_WARM = 10  # PE warmup matmuls (HAM un-throttle during DMA lead-in)

# NF=4 + linear fit of tanh(a+b) on [-10.5, 10.5], gaussian-weighted
# (fit.py): tanh(s) ~= ALPHA*s + sum c_f sin(w_f s); e2e rel err ~1e-2.
FREQS = [
    (0.5345, 0.57626),
    (1.0755, 0.19653),
    (1.6678, 0.11397),
    (2.7215, 0.03656),
]
ALPHA = 0.17034
NF = len(FREQS)
R = 2 * NF
NC_EXPBIAS = R          # ctab cols R..R+3: exp bias per jc
NC_BSIN = R + 4         # ctab col R+4: -pi/4 (sin atom bias)
NC_BCOS = R + 5         # ctab col R+5: +pi/4 (cos atom bias)
CTAB_C = R + 6
SHIFT = 0.125           # recenters atom args into the Sin spline's range


def _emit(nc, debug=False):
    queryT = nc.dram_tensor("queryT", [D, TQ], F16, kind="ExternalInput")
    keyT = nc.dram_tensor("keyT", [D, TV], F16, kind="ExternalInput")
    value16 = nc.dram_tensor("value16", [TV, D], F16, kind="ExternalInput")
    wua = nc.dram_tensor("wua", [2 * D, U], F16, kind="ExternalInput")
    ctab = nc.dram_tensor("ctab", [128, CTAB_C], F32, kind="ExternalInput")
    ascale = nc.dram_tensor("ascale", [U, 1], F16, kind="ExternalInput")
    ctx16 = nc.dram_tensor("ctx16", [TQ, D], F16, kind="ExternalOutput")
    env = dict(locals())

    with tile.TileContext(nc) as tc:
        _emit_body(nc, tc, env, debug)


def _emit_body(nc, tc, env, debug):
    queryT, keyT, value16, wua = (
        env["queryT"], env["keyT"], env["value16"], env["wua"]
    )
    ctab, ascale, ctx16 = env["ctab"], env["ascale"], env["ctx16"]

    with tc.tile_pool(name="const", bufs=1) as const:
        ctab_sb = const.tile([128, CTAB_C], F32, name="ctab_sb")
        ascale_sb = const.tile([U, 1], F16, name="ascale_sb")
        ones16 = const.tile([128, 1], F16, name="ones16")
        value_sb = const.tile([128, 4, D], F16, name="value_sb")
        qT16 = const.tile([U, TQ], F16, name="qT16")
        kT16 = const.tile([U, TV], F16, name="kT16")
        # atom argument tiles (shared by sin AND cos ACT insts), per side
        dk = const.tile([128, NF, TV], F16, name="dk")
        dq = const.tile([128, NF, TQ], F16, name="dq")
        # atom value tiles
        tk_s = const.tile([128, NF, TV], F16, name="tk_s")
        tk_c = const.tile([128, NF, TV], F16, name="tk_c")
        tq_s = const.tile([128, NF, TQ], F16, name="tq_s")
        tq_c = const.tile([128, NF, TQ], F16, name="tq_c")
        # wk = k-atoms * (scale_u * c_f) per term
        wkA = const.tile([128, NF, TV], F16, name="wkA")  # cos_k side
        wkB = const.tile([128, NF, TV], F16, name="wkB")  # sin_k side
        bias_sb = const.tile([128, 4], F32, name="bias_sb")
        wmT_sb = const.tile([128, 4, TQ], F16, name="wmT_sb")
        z_sb = const.tile([128, 4], F32, name="z_sb")
        zr_sb = const.tile([128, 4], F32, name="zr_sb")
        octx_sb = const.tile([128, 4, D], F16, name="octx_sb")
        scratch = const.tile([128, 2], F32, name="scratch")
        # f16 scratch for RR intermediates (r values per freq)
        rk = const.tile([128, NF, TV], F16, name="rk")
        rq = const.tile([128, NF, TQ], F16, name="rq")

        nc.vector.memset(ones16[:], 1.0)

        # ---- PE warmup during the DMA lead-in (HAM un-throttle) ----
        wsrc = const.tile([128, 512], F16, name="wsrc")
        nc.gpsimd.memset(wsrc[:], 0.0)
        # tiny dummy Sin: pin the sin table load into the DMA lead-in
        nc.scalar.activation(scratch[:, 0:1], wsrc[:, 0:1], AF.Sin, bias=0.0)
        with tc.tile_pool(name="warmps", bufs=1, space="PSUM") as warmps:
            wps = warmps.tile([128, 512], F32, name="wps")
            for _ in range(N_WARM):
                nc.tensor.matmul(wps[:], wsrc[:, :128], wsrc[:])

        # ---- input DMAs (order = arrival order; value16 last) ----
        qT_r = queryT.ap().rearrange("(c p) i -> p c i", p=128)
        kT_r = keyT.ap().rearrange("(c p) i -> p c i", p=128)
        wua_r = wua.ap().rearrange("(s c p) u -> p (s c) u", p=128, c=4)

        with (
            tc.tile_pool(name="projin", bufs=1) as projin,
            tc.tile_pool(name="projps", bufs=1, space="PSUM") as projps,
            tc.tile_pool(name="zkps", bufs=1, space="PSUM") as zkps,
        ):
            qin = projin.tile([128, 4, TQ], F16, name="qin")
            kin = projin.tile([128, 4, TV], F16, name="kin")
            wua_sb = projin.tile([128, 8, U], F16, name="wua_sb")
            # spread input DMA issues over two engine queues so transfers
            # start right after the iram preamble; value16 is issued late
            # (from the scalar queue, after the atom insts) so it doesn't
            # steal bandwidth from kin/qin
            nc.sync.dma_start(out=kin[:, 0:2, :], in_=kT_r[:, 0:2, :])
            nc.sync.dma_start(out=kin[:, 2:4, :], in_=kT_r[:, 2:4, :])
            nc.gpsimd.dma_start(out=wua_sb[:], in_=wua_r)
            nc.gpsimd.dma_start(out=ctab_sb[:], in_=ctab.ap())
            nc.gpsimd.dma_start(out=ascale_sb[:], in_=ascale.ap())
            # qin transfer is serialized behind kin (1-elem copy creates a
            # WAW dep) so kin gets the full early DMA bandwidth: the k-side
            # projection->args->Sin chain is the kernel's critical spine
            nc.gpsimd.tensor_copy(out=qin[0:1, 0, 0:1], in_=kin[0:1, 3, 0:1])
            nc.sync.dma_start(out=qin[:], in_=qT_r)

            # ---- projections (PE); casts to f16 ride ACT Copy ----
            qT_ps = projps.tile([U, TQ], F32, name="qT_ps")
            kT_ps = projps.tile([U, TV], F32, name="kT_ps")
            for dc in range(4):
                nc.tensor.matmul(
                    kT_ps[:], wua_sb[:, 4 + dc, :], kin[:, dc, :],
                    start=(dc == 0), stop=(dc == 3),
                )
            nc.scalar.activation(kT16[:], kT_ps[:], AF.Copy, bias=0.0)
            # zk[j] = (alpha*scale)^T @ kT16 per jc chunk (linear-term bias)
            zk_ps = zkps.tile([128, 4], F32, name="zk_ps")
            for jc in range(4):
                nc.tensor.matmul(
                    zk_ps[:, jc : jc + 1],
                    kT16[:, jc * 128 : (jc + 1) * 128],
                    ascale_sb[:],
                    start=True, stop=True,
                )
            for dc in range(4):
                nc.tensor.matmul(
                    qT_ps[:], wua_sb[:, dc, :], qin[:, dc, :],
                    start=(dc == 0), stop=(dc == 3),
                )
            nc.scalar.activation(qT16[:], qT_ps[:], AF.Copy, bias=0.0)
            # exp bias = host mask/shift bias + alpha*zk
            nc.vector.tensor_tensor(
                out=bias_sb[:], in0=ctab_sb[:, R : R + 4], in1=zk_ps[:],
                op=OP.add,
            )

        # ---- atom args (DVE) + atoms (ACT) + score matmuls (PE) ----
        def rr_group(src, d_t, r_t, fpair):
            """Range-reduction chains for two freqs of one side (DVE).
            d' = (x*w/2pi + SHIFT) - round(x*w/2pi + SHIFT); the +-pi/4 ACT
            biases then give sin/cos within the Sin spline's +-3.9 range."""
            for f in fpair:
                w2p = FREQS[f][0] / TWO_PI
                dsl = d_t[:, f, :]
                nc.vector.tensor_scalar(
                    dsl, src[:], w2p, SHIFT, OP.mult, OP.add
                )
                if f != 0:
                    rsl = r_t[:, f, :]
                    nc.vector.tensor_scalar(
                        rsl, dsl, MAGIC, MAGIC, OP.add, OP.subtract
                    )
                    nc.vector.tensor_tensor(
                        out=dsl, in0=dsl, in1=rsl, op=OP.subtract
                    )

        def act_atoms(d_t, out_t, fpair, kind):
            """One batched ACT Sin over a 2-freq group of one side."""
            col = NC_BSIN if kind == "sin" else NC_BCOS
            f0 = fpair[0]
            n = len(fpair)
            nc.scalar.activation(
                out_t[:, f0 : f0 + n, :], d_t[:, f0 : f0 + n, :], AF.Sin,
                bias=ctab_sb[:, col : col + 1], scale=TWO_PI,
            )

        def score_mms(wk_t, tq_t, fpair, start_f, stop_f, jc_outer=False):
            order = (
                [(f, jc) for jc in range(4) for f in fpair]
                if jc_outer
                else [(f, jc) for f in fpair for jc in range(4)]
            )
            for f, jc in order:
                nc.tensor.matmul(
                    sT_ps[jc][:],
                    wk_t[:, f, jc * 128 : (jc + 1) * 128],
                    tq_t[:, f, :],
                    start=(f == start_f), stop=(f == stop_f),
                )

        with (
            tc.tile_pool(name="spsum", bufs=1, space="PSUM") as spsum,
            tc.tile_pool(name="kwps", bufs=1, space="PSUM") as kwps,
        ):
            sT_ps = [
                spsum.tile([128, TQ], F32, name=f"sT_ps{jc}") for jc in range(4)
            ]
            kw_ps = kwps.tile([128, 512], F32, name="kw_ps")

            G0, G1 = (0, 1), (2, 3)
            # DVE arg order: k01, k23, q01, wkA01, q23, wkA23 (then wkB)
            rr_group(kT16, dk, rk, G0)
            act_atoms(dk, tk_c, G0, "cos")
            rr_group(kT16, dk, rk, G1)
            act_atoms(dk, tk_c, G1, "cos")
            rr_group(qT16, dq, rq, G0)
            for f in G0:
                nc.vector.tensor_scalar(
                    wkA[:, f, :], tk_c[:, f, :],
                    ctab_sb[:, 2 * f : 2 * f + 1], None, OP.mult,
                )
            act_atoms(dq, tq_s, G0, "sin")
            score_mms(wkA, tq_s, G0, start_f=0, stop_f=-1)
            rr_group(qT16, dq, rq, G1)
            for f in G1:
                nc.vector.tensor_scalar(
                    wkA[:, f, :], tk_c[:, f, :],
                    ctab_sb[:, 2 * f : 2 * f + 1], None, OP.mult,
                )
            # keepwarm MMs anchored on fresh args (prevent HAM re-throttle)
            nc.tensor.matmul(kw_ps[:], dk[:, 1, 0:128], wsrc[:],
                             start=True, stop=True)
            nc.tensor.matmul(kw_ps[:], dk[:, 3, 0:128], wsrc[:],
                             start=True, stop=True)
            nc.tensor.matmul(kw_ps[:], dq[:, 1, 0:128], wsrc[:],
                             start=True, stop=True)
            act_atoms(dq, tq_s, G1, "sin")
            score_mms(wkA, tq_s, G1, start_f=-1, stop_f=-1)
            # pin the value16 DMA behind the k-atom phase: the 1-elem copy
            # below depends on tk_c, and the DMA (WAW on value_sb) then
            # cannot be hoisted into the kin/qin transfer window
            nc.vector.tensor_copy(
                out=value_sb[0:1, 0, 0:1], in_=tk_c[0:1, 0, 0:1]
            )
            act_atoms(dk, tk_s, G0, "sin")
            act_atoms(dk, tk_s, G1, "sin")
            for f in range(NF):
                nc.vector.tensor_scalar(
                    wkB[:, f, :], tk_s[:, f, :],
                    ctab_sb[:, 2 * f + 1 : 2 * f + 2], None, OP.mult,
                )
            act_atoms(dq, tq_c, G0, "cos")
            score_mms(wkB, tq_c, G0, start_f=-1, stop_f=-1)
            act_atoms(dq, tq_c, G1, "cos")
            # value16 DMA: issued from the scalar queue, gated by the
            # tk_c-dependent 1-elem write above
            nc.gpsimd.dma_start(
                out=value_sb[:],
                in_=value16.ap().rearrange("(c p) d -> p c d", p=128),
            )
            # dummy Exp pinned after the last Sin: prefetch exp table
            nc.scalar.activation(
                scratch[:, 1:2], tq_c[:, 0, 0:1], AF.Exp, bias=0.0
            )
            score_mms(wkB, tq_c, G1, start_f=-1, stop_f=3, jc_outer=True)
            # exp with mask+shift+linear bias per partition
            for jc in range(4):
                nc.scalar.activation(
                    wmT_sb[:, jc, :], sT_ps[jc][:], AF.Exp,
                    bias=bias_sb[:, jc : jc + 1],
                )

        # ---- tail: ctx = wmT^T @ value, Z = wmT^T @ ones, normalize ----
        with (
            tc.tile_pool(name="cpsum", bufs=1, space="PSUM") as cpsum,
            tc.tile_pool(name="zpsum", bufs=1, space="PSUM") as zpsum,
        ):
            c_ps = [cpsum.tile([128, D], F32, name=f"c_ps{ic}") for ic in range(4)]
            z_ps = [zpsum.tile([128, 1], F32, name=f"z_ps{ic}") for ic in range(4)]
            # jc-outer: ctx MMs for jc fire right after exp(jc); in the
            # final jc round the z MMs go first so the z->recip->normalize
            # chain overlaps the remaining ctx MMs
            for jc in range(4):
                for ic in range(4):
                    lhsT = wmT_sb[:, jc, ic * 128 : (ic + 1) * 128]
                    if jc == 3:
                        nc.tensor.matmul(
                            z_ps[ic][:], lhsT, ones16[:],
                            start=False, stop=True,
                        )
                        nc.tensor.matmul(
                            c_ps[ic][:], lhsT, value_sb[:, jc, :],
                            start=False, stop=True,
                        )
                    else:
                        nc.tensor.matmul(
                            c_ps[ic][:], lhsT, value_sb[:, jc, :],
                            start=(jc == 0), stop=False,
                        )
                        nc.tensor.matmul(
                            z_ps[ic][:], lhsT, ones16[:],
                            start=(jc == 0), stop=False,
                        )
            for ic in range(4):
                nc.vector.tensor_copy(
                    out=z_sb[:, ic : ic + 1], in_=z_ps[ic][:]
                )
            nc.vector.reciprocal(out=zr_sb[:], in_=z_sb[:])
            # normalize: ics 0,2 on ACT (Copy w/ per-partition scale), 1,3 on
            # DVE, so the two halves run concurrently; 2 batched out-DMAs
            for pair in ((0, 1), (2, 3)):
                a, b = pair
                nc.scalar.activation(
                    octx_sb[:, a, :], c_ps[a][:], AF.Copy,
                    scale=zr_sb[:, a : a + 1],
                )
                nc.vector.tensor_scalar(
                    octx_sb[:, b, :], c_ps[b][:],
                    zr_sb[:, b : b + 1], None, OP.mult,
                )
                nc.sync.dma_start(
                    out=ctx16.ap().rearrange(
                        "(c p) d -> p c d", p=128
                    )[:, a : b + 1, :],
                    in_=octx_sb[:, a : b + 1, :],
                )


class _Runner:
    """Builds the Bass module once and holds a reusable jitted shard_map
    callable (mirrors concourse.bass2jax.run_bass_via_pjrt, but persistent
    so repeat calls don't re-jit/re-compile)."""

    def __init__(self, debug=False):
        import jax
        from concourse.bass2jax import install_neuronx_cc_hook, _bass_exec_p
        from jax.experimental.shard_map import shard_map
        from jax.sharding import Mesh, PartitionSpec

        self.jax = jax
        nc = bacc.Bacc(
            "TRN2", target_bir_lowering=False, debug=False,
            enable_asserts=False, num_devices=N_CORES,
            enable_partition_id=False,
        )
        _emit(nc, debug=debug)
        nc.compile()
        self.nc = nc

        install_neuronx_cc_hook()
        in_names, out_names, out_avals = [], [], []
        for alloc in nc.m.functions[0].allocations:
            if not isinstance(alloc, mybir.MemoryLocationSet):
                continue
            name = alloc.memorylocations[0].name
            if alloc.kind == "ExternalInput":
                in_names.append(name)
            elif alloc.kind == "ExternalOutput":
                out_names.append(name)
                out_avals.append(
                    jax.core.ShapedArray(
                        tuple(alloc.tensor_shape), mybir.dt.np(alloc.dtype)
                    )
                )
        assert nc.partition_id_tensor is None
        self.in_names = in_names
        self.out_names = out_names
        self.out_avals = out_avals
        n_params = len(in_names)
        n_outs = len(out_names)
        all_names = tuple(in_names + out_names)

        def _body(*args):
            outs = _bass_exec_p.bind(
                *args,
                out_avals=tuple(out_avals),
                in_names=all_names,
                out_names=tuple(out_names),
                lowering_input_output_aliases=(),
                sim_require_finite=True,
                sim_require_nnan=True,
                nc=nc,
            )
            return tuple(outs)

        devices = jax.devices()[:N_CORES]
        self.mesh = Mesh(np.asarray(devices), ("core",))
        self.pspec = PartitionSpec("core")
        in_specs = (self.pspec,) * (n_params + n_outs)
        out_specs = (self.pspec,) * n_outs
        donate = tuple(range(n_params, n_params + n_outs))
        self.sharded = jax.jit(
            shard_map(
                _body, mesh=self.mesh, in_specs=in_specs, out_specs=out_specs,
                check_rep=False,
            ),
            donate_argnums=donate,
            keep_unused=True,
        )

    def concat_inputs(self, in_maps):
        return [
            np.concatenate([np.asarray(m[name]) for m in in_maps], axis=0)
            for name in self.in_names
        ]

    def fresh_zeros(self):
        return [
            np.zeros((N_CORES * a.shape[0], *a.shape[1:]), a.dtype)
            for a in self.out_avals
        ]

    def run(self, in_maps):
        out_arrs = self.sharded(*self.concat_inputs(in_maps), *self.fresh_zeros())
        i = self.out_names.index("ctx16")
        a = self.out_avals[i]
        return (
            np.asarray(out_arrs[i])
            .reshape(N_CORES, *a.shape)
            .astype(np.float32)
        )


_runner = None


def _get_runner():
    global _runner
    if _runner is None:
        _runner = _Runner()
    return _runner


def _make_in_maps(query, key, value, mask, Wa, Ua, scale):
    query = np.asarray(query, dtype=np.float32)
    key = np.asarray(key, dtype=np.float32)
    value = np.asarray(value, dtype=np.float32)
    mask = np.asarray(mask)
    Wa = np.ascontiguousarray(np.asarray(Wa, dtype=np.float32))
    Ua = np.ascontiguousarray(np.asarray(Ua, dtype=np.float32))
    scale = np.ascontiguousarray(np.asarray(scale, dtype=np.float32))
    wua = np.concatenate([Wa, Ua], axis=0).astype(np.float16)
    ctab = np.zeros((128, CTAB_C), dtype=np.float32)
    for f, (_w, c) in enumerate(FREQS):
        ctab[:, 2 * f] = scale * c
        ctab[:, 2 * f + 1] = scale * c
    ctab[:, NC_BSIN] = -math.pi / 4
    ctab[:, NC_BCOS] = math.pi / 4
    ascale16 = (ALPHA * scale).astype(np.float16).reshape(U, 1)
    in_maps = []
    for b in range(B):
        mb = np.where(mask[b], EXP_SHIFT, EXP_SHIFT + MASK_NEG).astype(np.float32)
        ct = ctab.copy()
        ct[:, R : R + 4] = mb.reshape(4, 128).T
        in_maps.append(
            {
                "queryT": np.ascontiguousarray(query[b].T).astype(np.float16),
                "keyT": np.ascontiguousarray(key[b].T).astype(np.float16),
                "value16": value[b].astype(np.float16),
                "wua": wua,
                "ctab": ct,
                "ascale": ascale16,
            }
        )
    return in_maps


def kernel(query, key, value, mask, Wa, Ua, scale):
    r = _get_runner()
    in_maps = _make_in_maps(query, key, value, mask, Wa, Ua, scale)
    return r.run(in_maps)
